# revision 24
# baseline (speedup 1.0000x reference)
"""Bass/Tile TRN2 kernel for nn_AttnDecoder: attention decoder with LSTM cell.

Contract: kernel(**full_inputs) -> full output [B, S, OUT].
Shards batch B=128 over 8 NeuronCores (16 each), runs the sequential
scan fully on-device, gathers at the end.

v2: two phase-shifted batch groups of 8 per core. Each group's serial
tail (scores->exp->ctx->gates->LSTM->hc) hides under the other group's
big attention tanh, so the step cadence approaches ACT-busy instead of
chain latency.

Layouts (per core, BL=16 local batches, 2 groups x GW=8):
  - "^T" tensors put the feature dim on SBUF partitions, batch in free.
  - Big [*, cols] tensors use (h-half, s-major, b-minor) free order:
    col = h*SB + s*16 + b, so the per-batch hc broadcast adds keep a
    packed innermost dim (DVE 2x mode).
  - LSTM gate columns are host-permuted from [i,f,g,o] to [i,f,o,g], and
    the g-gate weights/bias host-doubled so one tanh(0.5*x) activation
    yields sigmoid-form for i,f,o and true tanh for g.
  - States are tracked doubled (H=2h, C=2c); the 0.5 is folded into
    Wh/Wc/Whh/fc_out_W host-side. Saves pointwise ops per step.
  - Softmax never normalized on the critical path: context matmuls
    consume exp(scores); the normalizer comes from a ones-matmul and is
    folded into the y_tilde combine.
  - Biases zero-cost folded: attn_b2 dropped (softmax invariant),
    lstm/fc biases ride as an extra ones-row in yT, fc_out_b on host.
"""

import numpy as np
import ml_dtypes

B, S, E, D, OUT = 128, 128, 256, 256, 64
NCORES, BL = 8, 16
G, GW = 2, 8  # groups per core, batches per group
import os as _os

NSTEPS = int(_os.environ.get("ATTN_NSTEPS", S))
SB = S * BL
GSB = S * GW  # 1024: per-group (b, s) cols per e-half
BF = ml_dtypes.bfloat16

_built = None  # cached program (input-independent)


def _host_prep(inputs):
    """Cast/transpose/permute everything on the host into device-ready arrays."""
    f32 = np.float32
    enc = np.ascontiguousarray(np.asarray(inputs["input_encoded"], f32))
    y = np.asarray(inputs["y_history"], f32)
    h0 = np.asarray(inputs["h0"], f32)
    c0 = np.asarray(inputs["c0"], f32)
    W1 = np.asarray(inputs["attn_W1"], f32)
    b1 = np.asarray(inputs["attn_b1"], f32)
    w2 = np.asarray(inputs["attn_w2"], f32)
    Wih = np.asarray(inputs["lstm_Wih"], f32)
    Whh = np.asarray(inputs["lstm_Whh"], f32)
    bg = np.asarray(inputs["lstm_bih"], f32) + np.asarray(inputs["lstm_bhh"], f32)
    fcW = np.asarray(inputs["fc_W"], f32)
    fcb = np.asarray(inputs["fc_b"], f32)
    foW = np.asarray(inputs["fc_out_W"], f32)

    # doubled states: H=2h, C=2c -> halve every consumer of h and c
    Wh = 0.5 * W1[:D]
    Wc = 0.5 * W1[D : 2 * D]
    We = W1[2 * D :]
    Whh = 0.5 * Whh
    foW = np.concatenate([0.5 * foW[:D], foW[D:]], axis=0)

    # [i,f,g,o] -> [i,f,o,g], then double the g gate (tanh(0.5*2x) == tanh(x))
    gp = np.concatenate([np.arange(0, 2 * D), np.arange(3 * D, 4 * D), np.arange(2 * D, 3 * D)])
    Wih_p, Whh_p, bg_p = Wih[:, gp].copy(), Whh[:, gp].copy(), bg[gp].copy()
    Wih_p[:, 3 * D :] *= 2.0
    Whh_p[:, 3 * D :] *= 2.0
    bg_p[3 * D :] *= 2.0

    shared = {
        "whcd": np.concatenate(
            [Wh[:128], Wh[128:], Wc[:128], Wc[128:]], axis=1
        ).astype(BF),  # [128, 4E]
        "b1d": b1.reshape(2, 128).T.copy().astype(f32),  # [128, 2]
        "wed": np.concatenate([We[:128], We[128:]], axis=1).astype(BF),  # [128, 2E]
        "w2d": w2.reshape(2, 128).T.copy().astype(BF),  # [128, 2]
        # gates y-head folded through fc: My = fcW_y @ Wih (+ fc/lstm biases
        # as a 65th row, driven by the ones row of yT)
        "wihd": Wih_p.astype(BF),  # [64, 1024]
        "myd": np.concatenate(
            [fcW[E:] @ Wih_p, (fcb @ Wih_p + bg_p)[None, :]], 0
        ).astype(BF),  # [65, 1024]
        "whhd": Whh_p.reshape(2, 128, 8, 128).transpose(1, 0, 2, 3).reshape(
            128, 16 * 128
        ).copy().astype(BF),  # [128, (k,m)*128]
        # ctx head of fc, with a ones column is NOT added (pz separate)
        "fccd": np.concatenate([fcW[:128], fcW[128:256]], axis=1).astype(BF),  # [128, 2*64]
        "woutd": np.concatenate(
            [foW[k * 128 : (k + 1) * 128] for k in range(4)], axis=1
        ).astype(BF),  # [128, 4*8192]
    }

    per_core = []
    for i in range(NCORES):
        sl = slice(i * BL, (i + 1) * BL)
        es = enc[sl]  # [16,S,E]
        ys = y[sl]  # [16,S,OUT]
        # states per group, doubled, cols (k, j)
        h0T = 2.0 * h0[sl].T.reshape(2, 128, BL).transpose(1, 0, 2)  # [128,2,16]
        c0T = 2.0 * c0[sl].T.reshape(2, 128, BL).transpose(1, 0, 2)
        m = {
            # enc^T: [e, s, b] -> [128, (k, s, b)] (e-in half major, s, b-minor)
            "encTd": es.transpose(2, 1, 0).reshape(2, 128, S * BL).transpose(1, 0, 2)
            .reshape(128, 2 * SB).copy().astype(BF),
            "encNd": es.transpose(1, 0, 2).reshape(128, BL * E).copy().astype(BF),  # [128(s), (b,e)]
            "yTd": np.concatenate(
                [ys.transpose(2, 1, 0).reshape(OUT, SB), np.ones((1, SB), f32)], 0
            ).astype(BF),  # [65, S*BL] (t-major, b-minor) + ones row
        }
        for g in range(G):
            gs = slice(g * GW, (g + 1) * GW)
            m[f"hc0d{g}"] = np.concatenate(
                [h0T[:, :, gs].reshape(128, 2 * GW), c0T[:, :, gs].reshape(128, 2 * GW)],
                axis=1,
            ).astype(BF)  # [128, 32]: H (k,j) | bf16(C) (k,j)
            m[f"c0Td{g}"] = c0T[:, :, gs].reshape(128, 2 * GW).copy().astype(f32)  # [128,16]
        m.update(shared)
        per_core.append(m)
    return per_core


def _build():
    global _built
    if _built is not None:
        return _built
    import concourse.mybir as mybir
    import concourse.tile as tile
    from concourse import bacc
    from contextlib import ExitStack

    dt = mybir.dt
    AF = mybir.ActivationFunctionType
    OP = mybir.AluOpType

    nc = bacc.Bacc("TRN2", target_bir_lowering=False, debug=False)

    # ---- DRAM I/O ----
    d_encT = nc.dram_tensor("encTd", [128, 2 * SB], dt.bfloat16, kind="ExternalInput")
    d_encN = nc.dram_tensor("encNd", [128, BL * E], dt.bfloat16, kind="ExternalInput")
    d_yT = nc.dram_tensor("yTd", [65, SB], dt.bfloat16, kind="ExternalInput")
    d_hc0 = [nc.dram_tensor(f"hc0d{g}", [128, 32], dt.bfloat16, kind="ExternalInput") for g in range(G)]
    d_c0T = [nc.dram_tensor(f"c0Td{g}", [128, 16], dt.float32, kind="ExternalInput") for g in range(G)]
    d_whc = nc.dram_tensor("whcd", [128, 4 * E], dt.bfloat16, kind="ExternalInput")
    d_b1 = nc.dram_tensor("b1d", [128, 2], dt.float32, kind="ExternalInput")
    d_we = nc.dram_tensor("wed", [128, 2 * E], dt.bfloat16, kind="ExternalInput")
    d_w2 = nc.dram_tensor("w2d", [128, 2], dt.bfloat16, kind="ExternalInput")
    d_wih = nc.dram_tensor("wihd", [64, 1024], dt.bfloat16, kind="ExternalInput")
    d_my = nc.dram_tensor("myd", [65, 1024], dt.bfloat16, kind="ExternalInput")
    d_whh = nc.dram_tensor("whhd", [128, 16 * 128], dt.bfloat16, kind="ExternalInput")
    d_fcc = nc.dram_tensor("fccd", [128, 2 * OUT], dt.bfloat16, kind="ExternalInput")
    d_wout = nc.dram_tensor("woutd", [128, 4 * OUT * S], dt.bfloat16, kind="ExternalInput")
    d_out = nc.dram_tensor("outd", [BL, OUT * S], dt.float32, kind="ExternalOutput")

    with tile.TileContext(nc) as tc, ExitStack() as ctx:
        P = ctx.enter_context(tc.tile_pool(name="persist", bufs=1))

        def load(shape, dtype, src):
            t = P.tile(shape, dtype, tag=f"ld{load.n}", name=f"ld{load.n}")
            load.n += 1
            nc.sync.dma_start(t[:], src)
            return t

        load.n = 0

        # ---- resident tensors (scan-critical inputs first, wout last) ----
        ones = P.tile([128, 128], dt.bfloat16, tag="ones", name="ones")
        nc.gpsimd.memset(ones[:], 1.0)
        weT = load([128, 2 * E], dt.bfloat16, d_we[:])
        encTt = load([128, 2 * SB], dt.bfloat16, d_encT[:])
        b1t = load([128, 2], dt.float32, d_b1[:])
        b1T = [b1t[:, h : h + 1] for h in range(2)]
        fccT = load([128, 2 * OUT], dt.bfloat16, d_fcc[:])
        fcc = [fccT[:, k * OUT : (k + 1) * OUT] for k in range(2)]
        hcs = [load([128, 32], dt.bfloat16, d_hc0[g][:]) for g in range(G)]
        cT = [load([128, 16], dt.float32, d_c0T[g][:]) for g in range(G)]
        whcT = load([128, 4 * E], dt.bfloat16, d_whc[:])
        w2t = load([128, 2], dt.bfloat16, d_w2[:])
        w2sb = [w2t[:, h : h + 1] for h in range(2)]
        yT = load([65, SB], dt.bfloat16, d_yT[:])
        whhT = load([128, 16 * 128], dt.bfloat16, d_whh[:])
        whh = [
            [whhT[:, (k * 8 + m) * 128 : (k * 8 + m + 1) * 128] for m in range(8)]
            for k in range(2)
        ]
        wihT = load([64, 1024], dt.bfloat16, d_wih[:])
        wih = [wihT[:, m * 128 : (m + 1) * 128] for m in range(8)]
        myT = load([65, 1024], dt.bfloat16, d_my[:])
        my = [myT[:, m * 128 : (m + 1) * 128] for m in range(8)]
        encNt = load([128, BL * E], dt.bfloat16, d_encN[:])
        woutT = load([128, 4 * OUT * S], dt.bfloat16, d_wout[:])

        # encp: [128, (h, s, b)] — e-out half major, s, b-minor
        encp = P.tile([128, 2 * SB], dt.bfloat16, tag="encp", name="encp")
        encp4 = encp[:].rearrange("p (h s b) -> p h s b", h=2, b=BL)
        encF = [P.tile([128, OUT], dt.bfloat16, tag=f"encF{b}", name=f"encF{b}") for b in range(BL)]
        ctxT = [P.tile([128, 16], dt.bfloat16, tag=f"ctxT{g}", name=f"ctxT{g}") for g in range(G)]

        PS = ctx.enter_context(tc.tile_pool(name="psum", bufs=1, space="PSUM"))

        # ---- PE warm-up: dense dummy matmuls while init DMAs stream.
        # HAM un-throttles (1.2->2.4 GHz) after ~3.4us of sustained PE busy;
        # the scan's own gaps are short enough to then stay warm. One long
        # accumulation chain — back-to-back, no WAW serialization.
        pw = PS.tile([128, 128], dt.float32, tag="gA", name="warm")
        NWARM = 96
        for w in range(NWARM):
            nc.tensor.matmul(pw[:], ones[:], ones[:], start=(w == 0),
                             stop=(w == NWARM - 1), skip_group_check=True)

        # ---- init phase: encp = We^T enc^T (+b1); encF[b] = enc_b @ fcW_c ----
        encT4 = encTt[:].rearrange("p (k s b) -> p k s b", k=2, b=BL)
        for h in range(2):
            for nkc in range(4):
                ps = PS.tile([128, 512], dt.float32, tag="gB", name="eproj")
                csl = slice(nkc * 512, (nkc + 1) * 512)
                for k in range(2):
                    nc.tensor.matmul(
                        ps[:],
                        weT[:, k * E + h * 128 : k * E + (h + 1) * 128],
                        encTt[:, k * SB + nkc * 512 : k * SB + (nkc + 1) * 512],
                        start=(k == 0),
                        stop=(k == 1),
                    )
                nc.vector.tensor_scalar(
                    encp[:, h * SB + nkc * 512 : h * SB + (nkc + 1) * 512],
                    ps[:], b1T[h], None, OP.add,
                )
        for b in range(BL):
            pf = PS.tile([128, OUT], dt.float32, tag="yA", name="ef")
            for k in range(2):
                nc.tensor.matmul(
                    pf[:], encT4[:, k, :, b], fcc[k], start=(k == 0), stop=(k == 1)
                )
            if b % 2 == 0:
                nc.vector.tensor_copy(encF[b][:], pf[:])
            else:
                nc.scalar.activation(encF[b][:], pf[:], AF.Copy)

        # bridge warm-up: keep the PE HAM window busy across the init->scan
        # transition and pipeline fill (a single >3.4us idle window would
        # re-throttle the PE to 1.2GHz for the whole scan). tile_wait_until
        # floors place each chain in the right stretch of the PE stream; the
        # banks chosen are ones whose first scan use comes later.
        for tag, floor_ms, n in (("yA", 0.013, 40), ("zB", 0.018, 40), ("yB", 0.023, 40)):
            with tc.tile_wait_until(floor_ms):
                pwb = PS.tile([128, 128], dt.float32, tag=tag, name=f"warm_{tag}")
                for w in range(n):
                    nc.tensor.matmul(pwb[:], ones[:], ones[:], start=(w == 0),
                                     stop=(w == n - 1), skip_group_check=True)

        # ---- the scan: two groups, software-pipelined ----
        sp = ctx.enter_context(tc.tile_pool(name="step", bufs=2))

        def ptile(tag, shape):
            return PS.tile(shape, dt.float32, tag=tag, name=tag)

        def phase1(g, t):
            """state -> hc proj -> pre-add -> big tanh; also opens gates accum."""
            tg = "AB"[g]
            # hc^T = Wh^T H + Wc^T C (0.5 folded host-side) + b1(folded in encp)
            # eh-split: cast/add/tanh for e-half 0 start after only 4 matmuls.
            # phc shares its group's pz bank (WAR gate on last step's recip is
            # never binding in steady state). At t=0, group B's phc instead
            # uses group A's pz bank: the WAR dependency on A's first recip
            # primes the two pipelines a half step apart — the static schedule
            # then carries the anti-phase for the whole scan. Without this the
            # scheduler runs the groups in lockstep and the step period
            # degrades to the serial chain latency.
            phc = ptile("zA" if (g == 1 and t == 0) else f"z{tg}", [128, 16])
            nmm = 0
            for eh in range(2):
                o = phc[:, eh * 8 : (eh + 1) * 8]
                for part, st in ((2, slice(16, 24)), (3, slice(24, 32)),
                                 (0, slice(0, 8)), (1, slice(8, 16))):
                    nc.tensor.matmul(
                        o,
                        whcT[:, part * E + eh * 128 : part * E + (eh + 1) * 128],
                        hcs[g][:, st],
                        start=(nmm == 0), stop=(nmm % 4 == 3),
                        skip_group_check=True,
                    )
                    nmm += 1
            # gates: Whh part opens the bank; my part rides along; wih closes in phase2
            pg = ptile(f"g{tg}", [128, 64])
            for m in range(8):
                o = pg[:, m * 8 : (m + 1) * 8]
                nc.tensor.matmul(o, whh[0][m], hcs[g][:, 0:8], start=(m == 0),
                                 stop=False, skip_group_check=True)
                nc.tensor.matmul(o, whh[1][m], hcs[g][:, 8:16], start=False,
                                 stop=False, skip_group_check=True)
            for m in range(8):
                nc.tensor.matmul(
                    pg[:, m * 8 : (m + 1) * 8], my[m],
                    yT[:, t * BL + g * GW : t * BL + (g + 1) * GW],
                    start=False, stop=False, skip_group_check=True,
                )
            # pre = encp + hc (broadcast per (h, j) over s, j-minor keeps the
            # innermost dim packed so the DVE adds run in 2x mode). The h1
            # half runs on the otherwise-idle GpSimd engine: the two adds
            # proceed in parallel and DVE's big-op queue stays short, so the
            # other group's small tail ops don't strand behind them.
            Tt = []
            for h in range(2):
                hcT = sp.tile([128, 8], dt.bfloat16, tag=f"hcT{tg}{h}", name=f"hcT{tg}{h}")
                nc.vector.tensor_copy(hcT[:], phc[:, h * 8 : (h + 1) * 8])
                pre = sp.tile([128, GSB], dt.bfloat16, tag=f"pre{tg}{h}", name=f"pre{tg}{h}")
                hcb = hcT[:, None, :].to_broadcast((128, S, GW))
                nc.vector.tensor_tensor(
                    pre[:].rearrange("p (s j) -> p s j", j=GW),
                    encp4[:, h, :, g * GW : (g + 1) * GW], hcb, OP.add,
                )
                T = sp.tile([128, GSB], dt.bfloat16, tag=f"T{tg}{h}", name=f"T{tg}{h}")
                nc.scalar.activation(T[:], pre[:], AF.Tanh)
                Tt.append(T)
            return pg, Tt

        def phase2(g, t, pg, Tt):
            """scores -> exp -> ctx/pz -> y_tilde -> gates tail -> LSTM cell."""
            tg = "AB"[g]
            # scores^T[s, j] = w2 . T[h][:, :, j]; h-outer so the h0 half can
            # issue while tanh(h1) is still running
            psc = ptile(f"s{tg}", [128, GW])
            Ts = [Tt[h][:].rearrange("p (s j) -> p s j", j=GW) for h in range(2)]
            for h in range(2):
                for j in range(GW):
                    nc.tensor.matmul(
                        psc[:, j : j + 1],
                        Ts[h][:, :, j],
                        w2sb[h],
                        start=(h == 0 and j == 0), stop=(h == 1),
                        skip_group_check=True,
                    )
            pT = sp.tile([128, GW], dt.bfloat16, tag=f"pT{tg}", name=f"pT{tg}")
            nc.scalar.activation(pT[:], psc[:], AF.Exp)
            # normalizer (broadcast to all partitions) then ctx head of y_tilde
            pz = ptile(f"z{tg}", [128, GW])
            nc.tensor.matmul(pz[:], ones[:], pT[:], start=True, stop=True,
                             skip_group_check=True)
            pyt = ptile(f"y{tg}", [OUT, GW])
            for j in range(GW):
                nc.tensor.matmul(
                    pyt[:, j : j + 1], encF[g * GW + j][:], pT[:, j : j + 1],
                    start=(j == 0), stop=(j == GW - 1), skip_group_check=True,
                )
            rzB = sp.tile([128, GW], dt.float32, tag=f"rz{tg}", name=f"rz{tg}")
            nc.vector.reciprocal(rzB[:], pz[:])
            ytld = sp.tile([OUT, GW], dt.bfloat16, tag=f"yt{tg}", name=f"yt{tg}")
            nc.vector.tensor_tensor(ytld[:], pyt[:], rzB[0:OUT, :], OP.mult)
            # gates tail
            for m in range(8):
                nc.tensor.matmul(
                    pg[:, m * 8 : (m + 1) * 8], wih[m], ytld[:],
                    start=False, stop=True, skip_group_check=True,
                )
            # LSTM cell. cols of thG: i 0:16, f 16:32, o 32:48, g 48:64.
            thG = sp.tile([128, 64], dt.float32, tag=f"th{tg}", name=f"th{tg}")
            nc.scalar.activation(thG[:], pg[:], AF.Tanh, scale=0.5)
            u = sp.tile([128, 16], dt.float32, tag=f"u{tg}", name=f"u{tg}")
            nc.vector.scalar_tensor_tensor(u[:], thG[:, 16:32], 1.0, cT[g][:], OP.add, OP.mult)
            v = sp.tile([128, 16], dt.float32, tag=f"v{tg}", name=f"v{tg}")
            nc.vector.scalar_tensor_tensor(v[:], thG[:, 0:16], 1.0, thG[:, 48:64], OP.add, OP.mult)
            # C_new = 0.5*u + v
            nc.vector.scalar_tensor_tensor(cT[g][:], u[:], 0.5, v[:], OP.mult, OP.add)
            tcn = sp.tile([128, 16], dt.float32, tag=f"tc{tg}", name=f"tc{tg}")
            nc.scalar.activation(tcn[:], cT[g][:], AF.Tanh, scale=0.5)
            nc.vector.tensor_copy(hcs[g][:, 16:32], cT[g][:])
            # H_new = (th_o + 1) * tanh(c)
            nc.vector.scalar_tensor_tensor(
                hcs[g][:, 0:16], thG[:, 32:48], 1.0, tcn[:], OP.add, OP.mult
            )
            if t == NSTEPS - 1:
                # full context: ctxT[g][:, eh*8+j] = enc_bj[:, eh].T @ pT, * rz
                pcx = ptile(f"y{tg}", [128, 16])
                for j in range(GW):
                    b = g * GW + j
                    for eh in range(2):
                        nc.tensor.matmul(
                            pcx[:, eh * 8 + j : eh * 8 + j + 1],
                            encNt[:, b * E + eh * 128 : b * E + (eh + 1) * 128],
                            pT[:, j : j + 1],
                            start=(j == 0 and eh == 0),
                            stop=(j == GW - 1 and eh == 1), skip_group_check=True,
                        )
                cx3 = ctxT[g][:].rearrange("p (e j) -> p e j", j=GW)
                nc.vector.tensor_tensor(
                    cx3, pcx[:].rearrange("p (e j) -> p e j", j=GW),
                    rzB[:, None, :].to_broadcast((128, 2, GW)), OP.mult,
                )

        # pipeline, with manual schedule control: tile_wait_until floors give
        # every phase an explicit slot in the scheduler's virtual timeline —
        # group B offset half a period from group A so each group's serial
        # tail overlaps the other group's big tanh block. Floors only shape
        # the static per-engine instruction ORDER (generous values are safe);
        # real time follows the real dependency chain. Without them the
        # readiness-driven list scheduler runs the groups in lockstep and the
        # step period degrades to the serial chain latency plus contention.
        P_MS, T2_MS = 0.012, 0.005
        live = {}

        def slot(g, t, ph):
            return t * P_MS + g * P_MS / 2 + (T2_MS if ph == 2 else 0.0)

        with tc.tile_wait_until(slot(0, 0, 1)):
            live[0] = phase1(0, 0)
        with tc.tile_wait_until(slot(0, 0, 2)):
            phase2(0, 0, *live[0])
        with tc.tile_wait_until(slot(1, 0, 1)):
            live[1] = phase1(1, 0)
        for t in range(1, NSTEPS):
            with tc.tile_wait_until(slot(1, t - 1, 2)):
                phase2(1, t - 1, *live[1])
            with tc.tile_wait_until(slot(0, t, 1)):
                live[0] = phase1(0, t)
            with tc.tile_wait_until(slot(0, t, 2)):
                phase2(0, t, *live[0])
            with tc.tile_wait_until(slot(1, t, 1)):
                live[1] = phase1(1, t)
        with tc.tile_wait_until(slot(1, NSTEPS - 1, 2)):
            phase2(1, NSTEPS - 1, *live[1])

        # ---- final projection: out = [H|ctx] @ fc_out_W (0.5 for H folded
        # host-side; fc_out_b added on host). 4-way PE column tiling: chunks
        # n..n+3 run concurrently on col-groups 0..3 of the array.
        xh = [P.tile([128, 16], dt.bfloat16, tag=f"xh{k}", name=f"xh{k}") for k in range(4)]
        for k in range(2):
            for g in range(G):
                nc.vector.tensor_copy(xh[k][:, g * GW : (g + 1) * GW],
                                      hcs[g][:, k * 8 : (k + 1) * 8])
                nc.vector.tensor_copy(xh[2 + k][:, g * GW : (g + 1) * GW],
                                      ctxT[g][:, k * 8 : (k + 1) * 8])
        # ctx parts are ready well before H; do them first in each chain
        korder = (2, 3, 0, 1)
        for r in range(4):
            pf = PS.tile([128, 512], dt.float32, tag=("gA" if r % 2 else "gB"), name="fin")
            for cg in range(4):
                n = r * 4 + cg
                o = pf[cg * 32 : cg * 32 + 16, :]
                for i, k in enumerate(korder):
                    nc.tensor.matmul(
                        o, xh[k][:],
                        woutT[:, k * OUT * S + n * 512 : k * OUT * S + (n + 1) * 512],
                        start=(i == 0), stop=(i == 3),
                        tile_position=(0, cg * 32), skip_group_check=True,
                    )
            for cg in range(4):
                n = r * 4 + cg
                ob = sp.tile([16, 512], dt.float32, tag="ob", name="ob", bufs=8)
                nc.vector.tensor_copy(ob[:], pf[cg * 32 : cg * 32 + 16, :])
                nc.sync.dma_start(d_out[:, n * 512 : (n + 1) * 512], ob[:])

    nc.compile()
    _built = nc
    return nc


def _install_ntff_hook():
    """antenv.axon_hooks is absent in this image; synthesize it from the
    boot script's ctypes NTFF driver so trace=True yields exec_time_ns."""
    import sys
    import types

    if "antenv.axon_hooks" in sys.modules:
        return
    try:
        sys.path.insert(0, "/root/.axon_site/trn_agent_boot")
        from trn_boot import _ntff_profile_via_ctypes  # type: ignore

        hook = _ntff_profile_via_ctypes("/opt/axon/libaxon_pjrt.so")
    except Exception:
        hook = None
    mod = types.ModuleType("antenv.axon_hooks")
    mod._hook = hook
    mod.get_axon_ntff_profile_hook = lambda: mod._hook
    mod.set_axon_ntff_profile_hook = lambda h: setattr(mod, "_hook", h)
    sys.modules["antenv.axon_hooks"] = mod


def _run(inputs, trace=False, tmpdir=None):
    from concourse.bass_utils import run_bass_kernel_spmd

    if trace:
        _install_ntff_hook()

    nc = _build()
    in_maps = _host_prep(inputs)
    res = run_bass_kernel_spmd(
        nc, in_maps, list(range(NCORES)), trace=trace, tmpdir=tmpdir
    )
    out = np.concatenate([r["outd"] for r in res.results], axis=0)  # [B, OUT*S]
    out = out + np.asarray(inputs["fc_out_b"], np.float32)[None, :]
    return out.reshape(B, S, OUT).astype(np.float32), res


def kernel(**inputs) -> np.ndarray:
    out, _ = _run(inputs, trace=False)
    return out


# revision 27
# speedup vs baseline: 1.2023x; 1.2023x over previous
"""Bass/Tile TRN2 kernel for nn_AttnDecoder: attention decoder with LSTM cell.

Contract: kernel(**full_inputs) -> full output [B, S, OUT].
Shards batch B=128 over 8 NeuronCores (16 each), runs the sequential
scan fully on-device, gathers at the end.

v2: two phase-shifted batch groups of 8 per core. Each group's serial
tail (scores->exp->ctx->gates->LSTM->hc) hides under the other group's
big attention tanh, so the step cadence approaches ACT-busy instead of
chain latency.

Layouts (per core, BL=16 local batches, 2 groups x GW=8):
  - "^T" tensors put the feature dim on SBUF partitions, batch in free.
  - Big [*, cols] tensors use (h-half, s-major, b-minor) free order:
    col = h*SB + s*16 + b, so the per-batch hc broadcast adds keep a
    packed innermost dim (DVE 2x mode).
  - LSTM gate columns are host-permuted from [i,f,g,o] to [i,f,o,g], and
    the g-gate weights/bias host-doubled so one tanh(0.5*x) activation
    yields sigmoid-form for i,f,o and true tanh for g.
  - States are tracked doubled (H=2h, C=2c); the 0.5 is folded into
    Wh/Wc/Whh/fc_out_W host-side. Saves pointwise ops per step.
  - Softmax never normalized on the critical path: context matmuls
    consume exp(scores); the normalizer comes from a ones-matmul and is
    folded into the y_tilde combine.
  - Biases zero-cost folded: attn_b2 dropped (softmax invariant),
    lstm/fc biases ride as an extra ones-row in yT, fc_out_b on host.
"""

import numpy as np
import ml_dtypes

B, S, E, D, OUT = 128, 128, 256, 256, 64
NCORES, BL = 8, 16
G, GW = 2, 8  # groups per core, batches per group
import os as _os

NSTEPS = int(_os.environ.get("ATTN_NSTEPS", S))
SB = S * BL
GSB = S * GW  # 1024: per-group (b, s) cols per e-half
BF = ml_dtypes.bfloat16

_built = None  # cached program (input-independent)


def _host_prep(inputs):
    """Cast/transpose/permute everything on the host into device-ready arrays."""
    f32 = np.float32
    enc = np.ascontiguousarray(np.asarray(inputs["input_encoded"], f32))
    y = np.asarray(inputs["y_history"], f32)
    h0 = np.asarray(inputs["h0"], f32)
    c0 = np.asarray(inputs["c0"], f32)
    W1 = np.asarray(inputs["attn_W1"], f32)
    b1 = np.asarray(inputs["attn_b1"], f32)
    w2 = np.asarray(inputs["attn_w2"], f32)
    Wih = np.asarray(inputs["lstm_Wih"], f32)
    Whh = np.asarray(inputs["lstm_Whh"], f32)
    bg = np.asarray(inputs["lstm_bih"], f32) + np.asarray(inputs["lstm_bhh"], f32)
    fcW = np.asarray(inputs["fc_W"], f32)
    fcb = np.asarray(inputs["fc_b"], f32)
    foW = np.asarray(inputs["fc_out_W"], f32)

    # doubled states: H=2h, C=2c -> halve every consumer of h and c
    Wh = 0.5 * W1[:D]
    Wc = 0.5 * W1[D : 2 * D]
    We = W1[2 * D :]
    Whh = 0.5 * Whh
    foW = np.concatenate([0.5 * foW[:D], foW[D:]], axis=0)

    # [i,f,g,o] -> [i,f,o,g], then double the g gate (tanh(0.5*2x) == tanh(x))
    gp = np.concatenate([np.arange(0, 2 * D), np.arange(3 * D, 4 * D), np.arange(2 * D, 3 * D)])
    Wih_p, Whh_p, bg_p = Wih[:, gp].copy(), Whh[:, gp].copy(), bg[gp].copy()
    Wih_p[:, 3 * D :] *= 2.0
    Whh_p[:, 3 * D :] *= 2.0
    bg_p[3 * D :] *= 2.0

    shared = {
        "whcd": np.concatenate(
            [Wh[:128], Wh[128:], Wc[:128], Wc[128:]], axis=1
        ).astype(BF),  # [128, 4E]
        "b1d": b1.reshape(2, 128).T.copy().astype(f32),  # [128, 2]
        "wed": np.concatenate([We[:128], We[128:]], axis=1).astype(BF),  # [128, 2E]
        "w2d": w2.reshape(2, 128).T.copy().astype(BF),  # [128, 2]
        # gates y-head folded through fc: My = fcW_y @ Wih (+ fc/lstm biases
        # as a 65th row, driven by the ones row of yT)
        "wihd": Wih_p.astype(BF),  # [64, 1024]
        "myd": np.concatenate(
            [fcW[E:] @ Wih_p, (fcb @ Wih_p + bg_p)[None, :]], 0
        ).astype(BF),  # [65, 1024]
        "whhd": Whh_p.reshape(2, 128, 8, 128).transpose(1, 0, 2, 3).reshape(
            128, 16 * 128
        ).copy().astype(BF),  # [128, (k,m)*128]
        # ctx head of fc, with a ones column is NOT added (pz separate)
        "fccd": np.concatenate([fcW[:128], fcW[128:256]], axis=1).astype(BF),  # [128, 2*64]
        "woutd": np.concatenate(
            [foW[k * 128 : (k + 1) * 128] for k in range(4)], axis=1
        ).astype(BF),  # [128, 4*8192]
    }

    per_core = []
    for i in range(NCORES):
        sl = slice(i * BL, (i + 1) * BL)
        es = enc[sl]  # [16,S,E]
        ys = y[sl]  # [16,S,OUT]
        # states per group, doubled, cols (k, j)
        h0T = 2.0 * h0[sl].T.reshape(2, 128, BL).transpose(1, 0, 2)  # [128,2,16]
        c0T = 2.0 * c0[sl].T.reshape(2, 128, BL).transpose(1, 0, 2)
        m = {
            # enc^T: [e, s, b] -> [128, (k, s, b)] (e-in half major, s, b-minor)
            "encTd": es.transpose(2, 1, 0).reshape(2, 128, S * BL).transpose(1, 0, 2)
            .reshape(128, 2 * SB).copy().astype(BF),
            "encNd": es.transpose(1, 0, 2).reshape(128, BL * E).copy().astype(BF),  # [128(s), (b,e)]
            "yTd": np.concatenate(
                [ys.transpose(2, 1, 0).reshape(OUT, SB), np.ones((1, SB), f32)], 0
            ).astype(BF),  # [65, S*BL] (t-major, b-minor) + ones row
        }
        for g in range(G):
            gs = slice(g * GW, (g + 1) * GW)
            m[f"hc0d{g}"] = np.concatenate(
                [h0T[:, :, gs].reshape(128, 2 * GW), c0T[:, :, gs].reshape(128, 2 * GW)],
                axis=1,
            ).astype(BF)  # [128, 32]: H (k,j) | bf16(C) (k,j)
            m[f"c0Td{g}"] = c0T[:, :, gs].reshape(128, 2 * GW).copy().astype(f32)  # [128,16]
        m.update(shared)
        per_core.append(m)
    return per_core


def _build():
    global _built
    if _built is not None:
        return _built
    import concourse.mybir as mybir
    import concourse.tile as tile
    from concourse import bacc
    from contextlib import ExitStack

    dt = mybir.dt
    AF = mybir.ActivationFunctionType
    OP = mybir.AluOpType

    nc = bacc.Bacc("TRN2", target_bir_lowering=False, debug=False)

    # ---- DRAM I/O ----
    d_encT = nc.dram_tensor("encTd", [128, 2 * SB], dt.bfloat16, kind="ExternalInput")
    d_encN = nc.dram_tensor("encNd", [128, BL * E], dt.bfloat16, kind="ExternalInput")
    d_yT = nc.dram_tensor("yTd", [65, SB], dt.bfloat16, kind="ExternalInput")
    d_hc0 = [nc.dram_tensor(f"hc0d{g}", [128, 32], dt.bfloat16, kind="ExternalInput") for g in range(G)]
    d_c0T = [nc.dram_tensor(f"c0Td{g}", [128, 16], dt.float32, kind="ExternalInput") for g in range(G)]
    d_whc = nc.dram_tensor("whcd", [128, 4 * E], dt.bfloat16, kind="ExternalInput")
    d_b1 = nc.dram_tensor("b1d", [128, 2], dt.float32, kind="ExternalInput")
    d_we = nc.dram_tensor("wed", [128, 2 * E], dt.bfloat16, kind="ExternalInput")
    d_w2 = nc.dram_tensor("w2d", [128, 2], dt.bfloat16, kind="ExternalInput")
    d_wih = nc.dram_tensor("wihd", [64, 1024], dt.bfloat16, kind="ExternalInput")
    d_my = nc.dram_tensor("myd", [65, 1024], dt.bfloat16, kind="ExternalInput")
    d_whh = nc.dram_tensor("whhd", [128, 16 * 128], dt.bfloat16, kind="ExternalInput")
    d_fcc = nc.dram_tensor("fccd", [128, 2 * OUT], dt.bfloat16, kind="ExternalInput")
    d_wout = nc.dram_tensor("woutd", [128, 4 * OUT * S], dt.bfloat16, kind="ExternalInput")
    d_out = nc.dram_tensor("outd", [BL, OUT * S], dt.float32, kind="ExternalOutput")

    with tile.TileContext(nc) as tc, ExitStack() as ctx:
        P = ctx.enter_context(tc.tile_pool(name="persist", bufs=1))

        def load(shape, dtype, src):
            t = P.tile(shape, dtype, tag=f"ld{load.n}", name=f"ld{load.n}")
            load.n += 1
            nc.sync.dma_start(t[:], src)
            return t

        load.n = 0

        # ---- resident tensors (scan-critical inputs first, wout last) ----
        ones = P.tile([128, 128], dt.bfloat16, tag="ones", name="ones")
        nc.gpsimd.memset(ones[:], 1.0)
        weT = load([128, 2 * E], dt.bfloat16, d_we[:])
        encTt = load([128, 2 * SB], dt.bfloat16, d_encT[:])
        b1t = load([128, 2], dt.float32, d_b1[:])
        b1T = [b1t[:, h : h + 1] for h in range(2)]
        fccT = load([128, 2 * OUT], dt.bfloat16, d_fcc[:])
        fcc = [fccT[:, k * OUT : (k + 1) * OUT] for k in range(2)]
        hcs = [load([128, 32], dt.bfloat16, d_hc0[g][:]) for g in range(G)]
        cT = [load([128, 16], dt.float32, d_c0T[g][:]) for g in range(G)]
        whcT = load([128, 4 * E], dt.bfloat16, d_whc[:])
        w2t = load([128, 2], dt.bfloat16, d_w2[:])
        w2sb = [w2t[:, h : h + 1] for h in range(2)]
        yT = load([65, SB], dt.bfloat16, d_yT[:])
        whhT = load([128, 16 * 128], dt.bfloat16, d_whh[:])
        whh = [
            [whhT[:, (k * 8 + m) * 128 : (k * 8 + m + 1) * 128] for m in range(8)]
            for k in range(2)
        ]
        wihT = load([64, 1024], dt.bfloat16, d_wih[:])
        wih = [wihT[:, m * 128 : (m + 1) * 128] for m in range(8)]
        myT = load([65, 1024], dt.bfloat16, d_my[:])
        my = [myT[:, m * 128 : (m + 1) * 128] for m in range(8)]
        encNt = load([128, BL * E], dt.bfloat16, d_encN[:])
        woutT = load([128, 4 * OUT * S], dt.bfloat16, d_wout[:])

        # encp: [128, (h, s, b)] — e-out half major, s, b-minor
        encp = P.tile([128, 2 * SB], dt.bfloat16, tag="encp", name="encp")
        encp4 = encp[:].rearrange("p (h s b) -> p h s b", h=2, b=BL)
        encF = [P.tile([128, OUT], dt.bfloat16, tag=f"encF{b}", name=f"encF{b}") for b in range(BL)]
        ctxT = [P.tile([128, 16], dt.bfloat16, tag=f"ctxT{g}", name=f"ctxT{g}") for g in range(G)]

        PS = ctx.enter_context(tc.tile_pool(name="psum", bufs=1, space="PSUM"))

        # ---- init phase: encp = We^T enc^T (+b1); encF[b] = enc_b @ fcW_c ----
        encT4 = encTt[:].rearrange("p (k s b) -> p k s b", k=2, b=BL)
        for h in range(2):
            for nkc in range(4):
                ps = PS.tile([128, 512], dt.float32, tag="gB", name="eproj")
                csl = slice(nkc * 512, (nkc + 1) * 512)
                for k in range(2):
                    nc.tensor.matmul(
                        ps[:],
                        weT[:, k * E + h * 128 : k * E + (h + 1) * 128],
                        encTt[:, k * SB + nkc * 512 : k * SB + (nkc + 1) * 512],
                        start=(k == 0),
                        stop=(k == 1),
                    )
                nc.vector.tensor_scalar(
                    encp[:, h * SB + nkc * 512 : h * SB + (nkc + 1) * 512],
                    ps[:], b1T[h], None, OP.add,
                )
        for b in range(BL):
            pf = PS.tile([128, OUT], dt.float32, tag="yA", name="ef")
            for k in range(2):
                nc.tensor.matmul(
                    pf[:], encT4[:, k, :, b], fcc[k], start=(k == 0), stop=(k == 1)
                )
            if b % 2 == 0:
                nc.vector.tensor_copy(encF[b][:], pf[:])
            else:
                nc.scalar.activation(encF[b][:], pf[:], AF.Copy)



        # ---- the scan: two groups, software-pipelined ----
        sp = ctx.enter_context(tc.tile_pool(name="step", bufs=2))

        def ptile(tag, shape):
            return PS.tile(shape, dt.float32, tag=tag, name=tag)

        def phase1(g, t):
            """state -> hc proj -> pre-add -> big tanh; also opens gates accum."""
            tg = "AB"[g]
            # hc^T = Wh^T H + Wc^T C (0.5 folded host-side) + b1(folded in encp)
            # eh-split: cast/add/tanh for e-half 0 start after only 4 matmuls.
            # phc shares its group's pz bank (WAR gate on last step's recip is
            # never binding in steady state). At t=0, group B's phc instead
            # uses group A's pz bank: the WAR dependency on A's first recip
            # primes the two pipelines a half step apart — the static schedule
            # then carries the anti-phase for the whole scan. Without this the
            # scheduler runs the groups in lockstep and the step period
            # degrades to the serial chain latency.
            phc = ptile("zA" if (g == 1 and t == 0) else f"z{tg}", [128, 16])
            nmm = 0
            for eh in range(2):
                o = phc[:, eh * 8 : (eh + 1) * 8]
                for part, st in ((2, slice(16, 24)), (3, slice(24, 32)),
                                 (0, slice(0, 8)), (1, slice(8, 16))):
                    nc.tensor.matmul(
                        o,
                        whcT[:, part * E + eh * 128 : part * E + (eh + 1) * 128],
                        hcs[g][:, st],
                        start=(nmm == 0), stop=(nmm % 4 == 3),
                        skip_group_check=True,
                    )
                    nmm += 1
            # gates: Whh part opens the bank; my part rides along; wih closes in phase2
            pg = ptile(f"g{tg}", [128, 64])
            for m in range(8):
                o = pg[:, m * 8 : (m + 1) * 8]
                nc.tensor.matmul(o, whh[0][m], hcs[g][:, 0:8], start=(m == 0),
                                 stop=False, skip_group_check=True)
                nc.tensor.matmul(o, whh[1][m], hcs[g][:, 8:16], start=False,
                                 stop=False, skip_group_check=True)
            for m in range(8):
                nc.tensor.matmul(
                    pg[:, m * 8 : (m + 1) * 8], my[m],
                    yT[:, t * BL + g * GW : t * BL + (g + 1) * GW],
                    start=False, stop=False, skip_group_check=True,
                )
            # pre = encp + hc (broadcast per (h, j) over s, j-minor keeps the
            # innermost dim packed so the DVE adds run in 2x mode). The h1
            # half runs on the otherwise-idle GpSimd engine: the two adds
            # proceed in parallel and DVE's big-op queue stays short, so the
            # other group's small tail ops don't strand behind them.
            Tt = []
            for h in range(2):
                hcT = sp.tile([128, 8], dt.bfloat16, tag=f"hcT{tg}{h}", name=f"hcT{tg}{h}")
                nc.vector.tensor_copy(hcT[:], phc[:, h * 8 : (h + 1) * 8])
                pre = sp.tile([128, GSB], dt.bfloat16, tag=f"pre{tg}{h}", name=f"pre{tg}{h}")
                hcb = hcT[:, None, :].to_broadcast((128, S, GW))
                nc.vector.tensor_tensor(
                    pre[:].rearrange("p (s j) -> p s j", j=GW),
                    encp4[:, h, :, g * GW : (g + 1) * GW], hcb, OP.add,
                )
                T = sp.tile([128, GSB], dt.bfloat16, tag=f"T{tg}{h}", name=f"T{tg}{h}")
                nc.scalar.activation(T[:], pre[:], AF.Tanh)
                Tt.append(T)
            return pg, Tt

        def phase2(g, t, pg, Tt):
            """scores -> exp -> ctx/pz -> y_tilde -> gates tail -> LSTM cell."""
            tg = "AB"[g]
            # scores^T[s, j] = w2 . T[h][:, :, j]; h-outer so the h0 half can
            # issue while tanh(h1) is still running
            psc = ptile(f"s{tg}", [128, GW])
            Ts = [Tt[h][:].rearrange("p (s j) -> p s j", j=GW) for h in range(2)]
            for h in range(2):
                for j in range(GW):
                    nc.tensor.matmul(
                        psc[:, j : j + 1],
                        Ts[h][:, :, j],
                        w2sb[h],
                        start=(h == 0 and j == 0), stop=(h == 1),
                        skip_group_check=True,
                    )
            pT = sp.tile([128, GW], dt.bfloat16, tag=f"pT{tg}", name=f"pT{tg}")
            nc.scalar.activation(pT[:], psc[:], AF.Exp)
            # normalizer (broadcast to all partitions) then ctx head of y_tilde
            pz = ptile(f"z{tg}", [128, GW])
            nc.tensor.matmul(pz[:], ones[:], pT[:], start=True, stop=True,
                             skip_group_check=True)
            pyt = ptile(f"y{tg}", [OUT, GW])
            for j in range(GW):
                nc.tensor.matmul(
                    pyt[:, j : j + 1], encF[g * GW + j][:], pT[:, j : j + 1],
                    start=(j == 0), stop=(j == GW - 1), skip_group_check=True,
                )
            rzB = sp.tile([128, GW], dt.float32, tag=f"rz{tg}", name=f"rz{tg}")
            nc.vector.reciprocal(rzB[:], pz[:])
            ytld = sp.tile([OUT, GW], dt.bfloat16, tag=f"yt{tg}", name=f"yt{tg}")
            nc.vector.tensor_tensor(ytld[:], pyt[:], rzB[0:OUT, :], OP.mult)
            # gates tail
            for m in range(8):
                nc.tensor.matmul(
                    pg[:, m * 8 : (m + 1) * 8], wih[m], ytld[:],
                    start=False, stop=True, skip_group_check=True,
                )
            # LSTM cell. cols of thG: i 0:16, f 16:32, o 32:48, g 48:64.
            thG = sp.tile([128, 64], dt.float32, tag=f"th{tg}", name=f"th{tg}")
            nc.scalar.activation(thG[:], pg[:], AF.Tanh, scale=0.5)
            u = sp.tile([128, 16], dt.float32, tag=f"u{tg}", name=f"u{tg}")
            nc.vector.scalar_tensor_tensor(u[:], thG[:, 16:32], 1.0, cT[g][:], OP.add, OP.mult)
            v = sp.tile([128, 16], dt.float32, tag=f"v{tg}", name=f"v{tg}")
            nc.vector.scalar_tensor_tensor(v[:], thG[:, 0:16], 1.0, thG[:, 48:64], OP.add, OP.mult)
            # C_new = 0.5*u + v
            nc.vector.scalar_tensor_tensor(cT[g][:], u[:], 0.5, v[:], OP.mult, OP.add)
            tcn = sp.tile([128, 16], dt.float32, tag=f"tc{tg}", name=f"tc{tg}")
            nc.scalar.activation(tcn[:], cT[g][:], AF.Tanh, scale=0.5)
            nc.vector.tensor_copy(hcs[g][:, 16:32], cT[g][:])
            # H_new = (th_o + 1) * tanh(c)
            nc.vector.scalar_tensor_tensor(
                hcs[g][:, 0:16], thG[:, 32:48], 1.0, tcn[:], OP.add, OP.mult
            )
            if t == NSTEPS - 1:
                # full context: ctxT[g][:, eh*8+j] = enc_bj[:, eh].T @ pT, * rz
                pcx = ptile(f"y{tg}", [128, 16])
                for j in range(GW):
                    b = g * GW + j
                    for eh in range(2):
                        nc.tensor.matmul(
                            pcx[:, eh * 8 + j : eh * 8 + j + 1],
                            encNt[:, b * E + eh * 128 : b * E + (eh + 1) * 128],
                            pT[:, j : j + 1],
                            start=(j == 0 and eh == 0),
                            stop=(j == GW - 1 and eh == 1), skip_group_check=True,
                        )
                cx3 = ctxT[g][:].rearrange("p (e j) -> p e j", j=GW)
                nc.vector.tensor_tensor(
                    cx3, pcx[:].rearrange("p (e j) -> p e j", j=GW),
                    rzB[:, None, :].to_broadcast((128, 2, GW)), OP.mult,
                )

        # pipeline, with manual schedule control: tile_wait_until floors give
        # every phase an explicit slot in the scheduler's virtual timeline —
        # group B offset half a period from group A so each group's serial
        # tail overlaps the other group's big tanh block. Floors only shape
        # the static per-engine instruction ORDER (generous values are safe);
        # real time follows the real dependency chain. Without them the
        # readiness-driven list scheduler runs the groups in lockstep and the
        # step period degrades to the serial chain latency plus contention.
        INIT_MS, P_MS, T2_MS = 0.016, 0.0078, 0.0036
        live = {}

        def slot(g, t, ph):
            return INIT_MS + t * P_MS + g * P_MS / 2 + (T2_MS if ph == 2 else 0.0)

        with tc.tile_wait_until(slot(0, 0, 1)):
            live[0] = phase1(0, 0)
        with tc.tile_wait_until(slot(0, 0, 2)):
            phase2(0, 0, *live[0])
        with tc.tile_wait_until(slot(1, 0, 1)):
            live[1] = phase1(1, 0)
        for t in range(1, NSTEPS):
            with tc.tile_wait_until(slot(1, t - 1, 2)):
                phase2(1, t - 1, *live[1])
            with tc.tile_wait_until(slot(0, t, 1)):
                live[0] = phase1(0, t)
            with tc.tile_wait_until(slot(0, t, 2)):
                phase2(0, t, *live[0])
            with tc.tile_wait_until(slot(1, t, 1)):
                live[1] = phase1(1, t)
        with tc.tile_wait_until(slot(1, NSTEPS - 1, 2)):
            phase2(1, NSTEPS - 1, *live[1])

        # ---- final projection: out = [H|ctx] @ fc_out_W (0.5 for H folded
        # host-side; fc_out_b added on host). 4-way PE column tiling: chunks
        # n..n+3 run concurrently on col-groups 0..3 of the array.
        xh = [P.tile([128, 16], dt.bfloat16, tag=f"xh{k}", name=f"xh{k}") for k in range(4)]
        for k in range(2):
            for g in range(G):
                nc.vector.tensor_copy(xh[k][:, g * GW : (g + 1) * GW],
                                      hcs[g][:, k * 8 : (k + 1) * 8])
                nc.vector.tensor_copy(xh[2 + k][:, g * GW : (g + 1) * GW],
                                      ctxT[g][:, k * 8 : (k + 1) * 8])
        # ctx parts are ready well before H; do them first in each chain
        korder = (2, 3, 0, 1)
        for r in range(4):
            pf = PS.tile([128, 512], dt.float32, tag=("gA" if r % 2 else "gB"), name="fin")
            for cg in range(4):
                n = r * 4 + cg
                o = pf[cg * 32 : cg * 32 + 16, :]
                for i, k in enumerate(korder):
                    nc.tensor.matmul(
                        o, xh[k][:],
                        woutT[:, k * OUT * S + n * 512 : k * OUT * S + (n + 1) * 512],
                        start=(i == 0), stop=(i == 3),
                        tile_position=(0, cg * 32), skip_group_check=True,
                    )
            for cg in range(4):
                n = r * 4 + cg
                ob = sp.tile([16, 512], dt.float32, tag="ob", name="ob", bufs=8)
                nc.vector.tensor_copy(ob[:], pf[cg * 32 : cg * 32 + 16, :])
                nc.sync.dma_start(d_out[:, n * 512 : (n + 1) * 512], ob[:])

    nc.compile()
    _built = nc
    return nc


def _install_ntff_hook():
    """antenv.axon_hooks is absent in this image; synthesize it from the
    boot script's ctypes NTFF driver so trace=True yields exec_time_ns."""
    import sys
    import types

    if "antenv.axon_hooks" in sys.modules:
        return
    try:
        sys.path.insert(0, "/root/.axon_site/trn_agent_boot")
        from trn_boot import _ntff_profile_via_ctypes  # type: ignore

        hook = _ntff_profile_via_ctypes("/opt/axon/libaxon_pjrt.so")
    except Exception:
        hook = None
    mod = types.ModuleType("antenv.axon_hooks")
    mod._hook = hook
    mod.get_axon_ntff_profile_hook = lambda: mod._hook
    mod.set_axon_ntff_profile_hook = lambda h: setattr(mod, "_hook", h)
    sys.modules["antenv.axon_hooks"] = mod


def _run(inputs, trace=False, tmpdir=None):
    from concourse.bass_utils import run_bass_kernel_spmd

    if trace:
        _install_ntff_hook()

    nc = _build()
    in_maps = _host_prep(inputs)
    res = run_bass_kernel_spmd(
        nc, in_maps, list(range(NCORES)), trace=trace, tmpdir=tmpdir
    )
    out = np.concatenate([r["outd"] for r in res.results], axis=0)  # [B, OUT*S]
    out = out + np.asarray(inputs["fc_out_b"], np.float32)[None, :]
    return out.reshape(B, S, OUT).astype(np.float32), res


def kernel(**inputs) -> np.ndarray:
    out, _ = _run(inputs, trace=False)
    return out
